# revision 9
# baseline (speedup 1.0000x reference)
"""Trainium2 Bass kernel for KANCell: relu(sum(relu(x))) over 2**25 fp32.

Data-parallel sharded reduction: the flat vector is split across 8
NeuronCores. Each core streams its 16 MiB shard HBM->SBUF in 2 MiB tiles
and fuses relu + per-partition partial sum in a single ScalarE activation
(accum_out) per tile, overlapped with the next tile's DMA. Per-core
partials come back to the host, which does the final (tiny) sum + ReLU.

Raw bass (no TileContext): all input DMAs share one semaphore so the
instruction-level sync-wait counts stay within walrus's limits.
"""

import numpy as np

N = 33554432  # 2**25
N_CORES = 8
PER_CORE = N // N_CORES  # 4194304 elements, 16 MiB fp32
P = 128  # SBUF partitions
F = 1024  # free-dim elements per tile -> [128, 1024] fp32 = 512 KiB
N_TILES = PER_CORE // (P * F)  # 32

_CACHED = {}


def _build_nc_iters(n_iters: int, f: int = F):
    """Build the per-core program with the pipeline body repeated n_iters
    times (n_iters>1 is used only for wall-clock slope benchmarking)."""
    key = (n_iters, f)
    if key in _CACHED:
        return _CACHED[key]

    import concourse.bass as bass
    import concourse.mybir as mybir

    n_tiles = PER_CORE // (P * f)
    nc = bass.Bass()

    x = nc.declare_dram_parameter("x", [PER_CORE], mybir.dt.float32, isOutput=False)
    out = nc.declare_dram_parameter(
        "partials", [P, n_tiles], mybir.dt.float32, isOutput=True
    )
    xv = x.rearrange("(n p f) -> n p f", p=P, f=f)

    from contextlib import ExitStack

    with ExitStack() as ctx:
        buf = ctx.enter_context(
            nc.sbuf_tensor([P, n_tiles * f], mybir.dt.float32)
        )
        accs = ctx.enter_context(nc.sbuf_tensor([P, n_tiles], mybir.dt.float32))
        # One completion semaphore per tile: sem_i >= 16 iff DMA i fully
        # landed. A single shared semaphore is NOT safe here — SDMA engines
        # skew, so a count of 16*(i+1) can be reached before DMA i+1's last
        # byte lands (observed as undercounted sums at 32 tiles in flight).
        in_sems = [
            ctx.enter_context(nc.semaphore(name=f"in_sem_{i}"))
            for i in range(n_tiles)
        ]
        act_sem = ctx.enter_context(nc.semaphore())
        out_sem = ctx.enter_context(nc.semaphore())
        block = ctx.enter_context(nc.Block())

        @block.sync
        def _(sync):
            for j in range(n_iters):
                for i in range(n_tiles):
                    if j > 0:
                        # WAR: iter j's DMA into tile i must wait for
                        # iter j-1's ACT on tile i to have consumed it.
                        sync.wait_ge(act_sem, (j - 1) * n_tiles + i + 1)
                    sync.dma_start(
                        out=buf[:, i * f : (i + 1) * f], in_=xv[i]
                    ).then_inc(in_sems[i], 16)
                sync.wait_ge(act_sem, (j + 1) * n_tiles)
                sync.dma_start(out=out[:], in_=accs[:]).then_inc(out_sem, 16)
            sync.wait_ge(out_sem, 16 * n_iters)

        @block.scalar
        def _(scalar):
            for j in range(n_iters):
                if j > 0:
                    # WAR: accs is re-written each iter; previous gather
                    # DMA must have read it.
                    scalar.wait_ge(out_sem, 16 * j)
                for i in range(n_tiles):
                    scalar.wait_ge(in_sems[i], 16 * (j + 1))
                    # in-place relu; per-partition tile sum -> accs[:, i]
                    nc.scalar.activation(
                        buf[:, i * f : (i + 1) * f],
                        buf[:, i * f : (i + 1) * f],
                        mybir.ActivationFunctionType.Relu,
                        accum_out=accs[:, i : i + 1],
                    ).then_inc(act_sem, 1)

    _CACHED[key] = nc
    return nc


def _build_nc():
    return _build_nc_iters(1)


def kernel(x: np.ndarray) -> np.ndarray:
    from concourse.bass_utils import run_bass_kernel_spmd

    nc = _build_nc()

    x = np.ascontiguousarray(np.asarray(x, dtype=np.float32).reshape(-1))
    shards = x.reshape(N_CORES, PER_CORE)
    in_maps = [{"x": shards[i]} for i in range(N_CORES)]
    res = run_bass_kernel_spmd(nc, in_maps, list(range(N_CORES)))

    partials = np.stack([r["partials"] for r in res.results])  # [8, P, n_tiles]
    total = partials.astype(np.float64).sum()
    return np.asarray(max(total, 0.0), dtype=np.float32)
